# revision 19
# baseline (speedup 1.0000x reference)
"""MoE FFN (top-2 routing, 8 experts) on 8 Trainium2 NeuronCores.

Strategy (expert parallelism, per the sharding hint):
  - Host computes router logits / top-2 / softmax (tiny: T x E) and
    dispatches tokens: expert e's tokens are gathered into a padded
    [H, C] batch for core e (C = common capacity).
  - Core e runs the dense FFN for its expert on its gathered tokens:
        yT = ( GELU_tanh(x @ W1 + b1) @ W2 + b2 )^T
    computed fully transposed ([F,c] then [H,c]) so both matmuls use
    the weights as the stationary operand and no on-device transposes
    are needed. Matmul operands are fp16 (same PE rate as bf16 but 8x
    finer mantissa); accumulation is fp32 in PSUM.
  - The per-token combine weight is applied on the HOST during the
    scatter-add (cheap, and skips a whole [H, C] vector pass + the
    wtb upload on device).

Performance notes (v8, vs the ~152 us v3):
  - Measured exec_time anatomy (per core): exec spans from the first
    engine instruction (~5.6 us, framework pool-init memsets) to the
    end of a fixed ~8-9 us semaphore-teardown loop that follows the
    last DMA.  In between: DMA ring flow from ~7.8 us, first real
    matmul at ~12.65 us (gated by x0 + W1 piece delivery), a
    near-perfect matmul stream (~131.6 us busy, 1-2.5 us of gaps; the
    pure-streaming floor for C=1068 at fp16 is 128.2 us), and a
    ~2.0 us act->enqueue->store tail.  Matmuls run at full 2.4 GHz
    (N/2.4 + 2.5 ns each); LDWEIGHTS is hidden at N >= ~178.
  - The startup is AGGREGATE-DMA-bandwidth-walled: all 8 cores pull
    the same schedule simultaneously and a core only sees ~200-250
    GB/s total across its queues in the early window.  Attempts to
    start the PE earlier (x-strip splits, finer W1 head pieces, a
    third scalar-hwdge queue, re-balancing pieces between queues) all
    measured SLOWER on the worst core: per-queue shares drop when
    more queues are active, per-core delivery variance widens the
    8-core max, and any PE stall > ~1.5 us risks a HAM clock
    re-throttle that amplifies it.  The v3 two-queue plan feeds
    phase A at exactly the deliverable rate and is kept verbatim.
  - Last-chunk tail fix (the one kept change): the final output
    row-tile is computed in two half-width column groups; the second
    half accumulates into a *fresh* PSUM tile (psA pool slot, free by
    then) so its matmuls don't serialize behind the first half's
    activation through the whole-tile PSUM dependency (was a 547 ns
    PE stall right at the kernel tail).
  - Run-to-run variance: the graded max-over-8-cores number moves
    +-1.5 us with per-core DMA/HAM luck, and occasionally a run lands
    in the chip's P0 power state (every matmul at exactly 2.0 GHz
    instead of 2.4 -> ~178 us total, zero pipeline gaps).  P0 episodes
    correlate with back-to-back heavy runs and are not controllable
    from the kernel.  The exit-time semaphore teardown (~57-59
    EVENT_SEMAPHOREs per engine) is a fixed range sweep, NOT
    proportional to tile/tag count - consolidating tiles gains
    nothing there and risks WAW-serializing the W1 DMA stream.

Self-contained: hardcodes the problem shapes (H=768, F=3072, E=8, K=2).
"""

import os
import time

import numpy as np

H = 768
F = 3072
E = 8
K = 2
N_CORES = 8
P = 128
FM = F // P   # 24 f row-tiles
HK = H // P   # 6 contraction tiles for x@W1
HN = H // P   # 6 output row-tiles of yT
W2G = 4       # fk tiles per W2 DMA piece
N_W2P = FM // W2G

PRECISION = os.environ.get("MOE_PRECISION", "fp16")  # "fp16" | "bf16" | "fp32"
WARMUP_MM = int(os.environ.get("MOE_WARMUP_MM", "14"))

# W1 column pieces (fm-consumption order): a small head piece so the
# first matmul can start early, then 256-col pieces.
W1_PIECES = [(0, P)] + [(P + 256 * i, 256) for i in range(11)] + [(F - P, P)]


def _w1_piece_of(fm):
    if fm == 0:
        return 0, 0
    return (fm + 1) // 2, (0 if fm % 2 == 1 else P)


def _chunks(C):
    """Split C columns into chunks of width <= 512 (PSUM bank limit).

    The first chunk is biased slightly wider: it is consumed while W1 is
    still streaming in, and a wider chunk consumes W1 pieces more slowly.
    """
    n = max(1, -(-C // 512))
    if n == 1:
        return [(0, C)]
    w0 = min(512, ((-(-C // n) + 28) // 2) * 2)
    rest = C - w0
    m = n - 1
    base, rem = divmod(rest, m)
    ws = [w0] + [base + 1] * rem + [base] * (m - rem)
    out, c0 = [], 0
    for w in ws:
        out.append((c0, w))
        c0 += w
    return out


def _strips(chunks):
    """Per-chunk x DMA spans. Splitting chunk 0 into strips was tried
    (v4/v5) and measured SLOWER: the early window is aggregate-DMA-
    bandwidth-walled (~200-250 GB/s across all queues while all 8 cores
    pull), so starting the PE earlier just moves the stalls later.
    Single span per chunk."""
    return [[(0, w)] for _, w in chunks]


# ---------------------------------------------------------------------------
# Bass/Tile device kernel
# ---------------------------------------------------------------------------

def _build_bass(C, precision=None):
    from contextlib import ExitStack

    import concourse.bass as bass  # noqa: F401
    import concourse.tile as tile
    from concourse import bacc, mybir
    from concourse._compat import with_exitstack

    precision = precision or PRECISION
    f32 = mybir.dt.float32
    mdt = {"bf16": mybir.dt.bfloat16, "fp16": mybir.dt.float16,
           "fp32": f32}[precision]

    chunks = _chunks(C)
    strips = _strips(chunks)
    WMAX = max(w for _, w in chunks)

    nc = bacc.Bacc("TRN2", target_bir_lowering=False, debug=False,
                   num_devices=N_CORES)
    # All inputs are host-packed per-partition images (see _make_in_maps):
    # a DMA is always dst_tile[:] <- img[:, a:b] with contiguous rows.
    xgt = nc.dram_tensor("xgt", [P, HK * C], mdt, kind="ExternalInput").ap()
    w1 = nc.dram_tensor("w1", [P, HK * F], mdt, kind="ExternalInput").ap()
    w2 = nc.dram_tensor("w2", [P, FM * H], mdt, kind="ExternalInput").ap()
    cpk = nc.dram_tensor("cpk", [P, FM + HN], f32,
                         kind="ExternalInput").ap()
    y = nc.dram_tensor("y", [P, HN * C], mdt, kind="ExternalOutput").ap()

    gelu = mybir.ActivationFunctionType.Gelu_apprx_tanh
    ident = mybir.ActivationFunctionType.Identity

    @with_exitstack
    def body(ctx: ExitStack, tc: tile.TileContext):
        const = ctx.enter_context(tc.tile_pool(name="const", bufs=1))
        w1pool = ctx.enter_context(tc.tile_pool(name="w1pool", bufs=1))
        w2pool = ctx.enter_context(tc.tile_pool(name="w2pool", bufs=1))
        xp = ctx.enter_context(tc.tile_pool(name="xp", bufs=1))
        hp = ctx.enter_context(tc.tile_pool(name="hp", bufs=1))
        yp = ctx.enter_context(tc.tile_pool(name="yp", bufs=3))
        psAp = ctx.enter_context(tc.tile_pool(name="psA", bufs=2, space="PSUM"))
        psBp = ctx.enter_context(tc.tile_pool(name="psB", bufs=1, space="PSUM"))

        # --- PE warmup: ramp the HAM clock gate 1.2 -> 2.4 GHz during the
        # DMA-bound startup (needs >~3.4us of continuous PE busy; the clock
        # stays up once ramped). Sized to end ~when the first data lands.
        wtile = xp.tile([P, 512], mdt, tag="warm", name="warm")
        nc.vector.memset(wtile[:], 0.0)
        wps = psBp.tile([P, WMAX], f32, tag="psB0", name="warmps")
        for i in range(WARMUP_MM):
            nc.tensor.matmul(wps[:, :WMAX], lhsT=wtile[:, 0:P],
                             rhs=wtile[:, 0:WMAX],
                             start=(i == 0), stop=(i == WARMUP_MM - 1))

        # --- SBUF tiles
        b12 = const.tile([P, FM + HN], f32, name="b12")
        b1s = b12[:, 0:FM]
        b2s = b12[:, FM:]
        w1t = [w1pool.tile([P, HK, wdt], mdt, tag=f"w1p{i}", name=f"w1p{i}")
               for i, (_, wdt) in enumerate(W1_PIECES)]
        w2t = [w2pool.tile([P, W2G, H], mdt, tag=f"w2p{g}", name=f"w2p{g}")
               for g in range(N_W2P)]
        # one x tile per (chunk, strip) span
        xgs = []
        for ci, (c0, w) in enumerate(chunks):
            xgs.append([xp.tile([P, HK, sw], mdt, tag=f"xg{ci}_{si}",
                                name=f"xg{ci}_{si}")
                        for si, (s0, sw) in enumerate(strips[ci])])

        # --- DMA staging. Two rings (sync hwdge + gpsimd) share the HBM
        # pipe (~230-250 GB/s aggregate early while all 8 cores pull);
        # jobs are enqueued in consumption order per ring. Experiments
        # with a third (scalar hwdge) queue, x-strip splits, or finer W1
        # head pieces all measured SLOWER: the early window is aggregate-
        # bandwidth-walled and the Tile scheduler's DMA-timing sim must
        # stay consistent with reality or it interleaves phase-B matmuls
        # into phase-A against late W2 data. Swapping pieces between the
        # queues also measured slower on the worst core (per-core DMA
        # variance + HAM re-throttle amplification of >1.5us stalls), so
        # this keeps the original two-queue plan. The SCALAR engine must
        # stay DMA-free: it runs the phase-A activations from ~t0.
        def w1_dma(ring, i):
            c0, wdt = W1_PIECES[i]
            ring.dma_start(w1t[i][:], w1[:, HK * c0:HK * (c0 + wdt)])

        def xg_dma(ring, ci, si):
            c0, w = chunks[ci]
            s0, sw = strips[ci][si]
            a = c0 + s0
            ring.dma_start(xgs[ci][si][:], xgt[:, HK * a:HK * (a + sw)])

        def w2_dma(ring, g):
            ring.dma_start(w2t[g][:], w2[:, g * W2G * H:(g + 1) * W2G * H])

        xg_dma(nc.sync, 0, 0)
        w1_dma(nc.gpsimd, 0)
        nc.gpsimd.dma_start(b12[:], cpk[:])
        w1_dma(nc.sync, 1)
        for i in range(2, len(W1_PIECES)):
            w1_dma(nc.gpsimd if i % 2 == 0 else nc.sync, i)
        for ci in range(1, len(chunks)):
            xg_dma(nc.sync if ci % 2 == 1 else nc.gpsimd, ci, 0)
        for g in range(N_W2P):
            w2_dma(nc.gpsimd if g % 2 == 0 else nc.sync, g)

        def w1_tile(hk, fm):
            i, off = _w1_piece_of(fm)
            return w1t[i][:, hk, off:off + P]

        def w2_tile(fk, hn):
            return w2t[fk // W2G][:, fk % W2G, hn * P:(hn + 1) * P]

        psAs = [psAp.tile([P, WMAX], f32, tag="psA", name=f"psA{j}")
                for j in range(2)]
        psBs = [psBp.tile([P, WMAX], f32, tag=f"psB{j}", name=f"psB{j}")
                for j in range(HN)]
        htst = hp.tile([P, FM, WMAX], mdt, tag="hts", name="hts")
        yos = [yp.tile([P, HN, WMAX], mdt, tag="yout", name=f"yout{j}")
               for j in range(3)]

        pending_epi = []
        gi = 0  # psA round-robin across all phase-A groups
        for ci, (c0, w) in enumerate(chunks):
            last = ci == len(chunks) - 1
            # ---- phase A: hT[f, c] = gelu((x@W1)[c, f] + b1[f]) ----
            hts = htst
            # group order: for chunk 0, interleave (fm, strip) pairs to
            # match the DMA arrival order (fm0a, fm1a, fm0b, fm1b, 2a,
            # 2b, 3a, 3b, ...); later chunks have a single strip.
            if len(strips[ci]) > 1:
                # strip-a groups for fm0..3 first (their W1 pieces land
                # ~0.9us apart on the fast queue while x strip b is still
                # in flight on the slow one), then the b strips, then
                # (a, b) pairs per fm.
                order = [(fm, 0) for fm in range(4)]
                order += [(fm, 1) for fm in range(4)]
                order += [(fm, si) for fm in range(4, FM) for si in (0, 1)]
            else:
                order = [(fm, 0) for fm in range(FM)]
            for oi, (fm, si) in enumerate(order):
                s0, sw = strips[ci][si]
                ps = psAs[gi % 2]
                gi += 1
                for hk in range(HK):
                    nc.tensor.matmul(
                        ps[:, :sw],
                        lhsT=w1_tile(hk, fm),
                        rhs=xgs[ci][si][:, hk, :sw],
                        start=(hk == 0), stop=(hk == HK - 1),
                    )
                nc.scalar.activation(hts[:, fm, s0:s0 + sw], ps[:, :sw],
                                     gelu, bias=b1s[:, fm:fm + 1])
                if oi == 1 and pending_epi:
                    # previous chunk's deferred phase-B epilogue: emitted
                    # after this chunk's first two A-activations so the
                    # scalar engine frees psA slots without a PE stall.
                    for f in pending_epi:
                        f()
                    pending_epi = []

            # ---- phase B: yT[h, c] = sum_f W2[f, h] * hT[f, c] (+b2) ----
            yo = yos[ci % 3]
            if ci == 0 and not last:
                # fk-outer across 6 PSUM banks: W2[fk] is consumed
                # progressively, so its DMA can stream during the phase.
                for fk in range(FM):
                    for hn in range(HN):
                        nc.tensor.matmul(
                            psBs[hn][:, :w],
                            lhsT=w2_tile(fk, hn),
                            rhs=hts[:, fk, :w],
                            start=(fk == 0), stop=(fk == FM - 1),
                        )
                def _epi(yo=yo, c0=c0, w=w):
                    for hn in range(HN):
                        nc.scalar.activation(yo[:, hn, :w], psBs[hn][:, :w],
                                             ident, bias=b2s[:, hn:hn + 1])
                    nc.sync.dma_start(y[:, HN * c0:HN * (c0 + w)],
                                      yo[:, :, :w])
                pending_epi.append(_epi)
            else:
                # hn-outer: epilogue + store of each row-tile overlap the
                # remaining matmuls (short kernel tail on the last chunk).
                for hn in range(HN):
                    # On the very last output group, compute/store in two
                    # half-width column sub-groups so the final activation
                    # and store are half-size and overlap the first half's
                    # matmuls (shorter kernel tail). The second half gets a
                    # fresh PSUM tile (psA slot, free by now) so its
                    # matmuls don't serialize behind the first half's
                    # activation.
                    if last and hn == HN - 1:
                        groups = [((0, w // 2), psBs[hn]),
                                  ((w // 2, w),
                                   psAp.tile([P, WMAX], f32, tag="psA",
                                             name="pstail"))]
                    else:
                        groups = [((0, w), psBs[hn])]
                    for (a, b), ps in groups:
                        for fk in range(FM):
                            nc.tensor.matmul(
                                ps[:, a:b],
                                lhsT=w2_tile(fk, hn),
                                rhs=hts[:, fk, a:b],
                                start=(fk == 0), stop=(fk == FM - 1),
                            )
                        # (A split epilogue - parallel scalar-act/vector-add
                        # + dual-queue stores - was tried and measured
                        # SLOWER: writes to disjoint ranges of one tile are
                        # WAW-ordered, serializing the two acts, and the
                        # gpsimd store ring has ~2x the sync ring latency.)
                        nc.scalar.activation(yo[:, hn, a:b], ps[:, a:b],
                                             ident, bias=b2s[:, hn:hn + 1])
                        if last:
                            nc.sync.dma_start(
                                y[:, HN * c0 + hn * w + a:
                                  HN * c0 + hn * w + b],
                                yo[:, hn, a:b])
                if not last:
                    nc.sync.dma_start(y[:, HN * c0:HN * (c0 + w)],
                                      yo[:, :, :w])

    with tile.TileContext(nc) as tc:
        body(tc)
    nc.compile()
    return nc


# ---------------------------------------------------------------------------
# Host-side routing + dispatch
# ---------------------------------------------------------------------------

def _route(xf, gate_w):
    """Top-2 router in float64 for a numerically robust top-k set."""
    logits = xf.astype(np.float64) @ gate_w.astype(np.float64)  # [T, E]
    top_idx = np.argpartition(logits, E - K, axis=1)[:, E - K:]  # [T, K]
    top_val = np.take_along_axis(logits, top_idx, axis=1)
    m = top_val.max(axis=1, keepdims=True)
    ex = np.exp(top_val - m)
    wts = ex / ex.sum(axis=1, keepdims=True)  # [T, K] float64

    toks, ws = [], []
    for e in range(E):
        mask = top_idx == e  # [T, K]
        rows = np.nonzero(mask.any(axis=1))[0]
        toks.append(rows)
        ws.append(wts[mask].astype(np.float32))
    return toks, ws


def _np_mdt():
    import ml_dtypes
    return {"bf16": ml_dtypes.bfloat16, "fp16": np.float16,
            "fp32": np.float32}[PRECISION]


def _pack_w1(W1e, mdt):
    """[H, F] -> [P, HK*F] image matching the w1 piece tiles."""
    w = np.asarray(W1e, np.float32).astype(mdt).reshape(HK, P, F)
    cols = [w[:, :, c0:c0 + wd].transpose(1, 0, 2).reshape(P, HK * wd)
            for c0, wd in W1_PIECES]
    return np.ascontiguousarray(np.concatenate(cols, axis=1))


def _pack_w2(W2e, mdt):
    """[F, H] -> [P, FM*H] image (fk-major blocks)."""
    w = np.asarray(W2e, np.float32).astype(mdt).reshape(FM, P, H)
    return np.ascontiguousarray(w.transpose(1, 0, 2).reshape(P, FM * H))


def _pack_xg(xT, chunks, strips, mdt):
    """[H, C] -> [P, HK*C] image (span-major, hk-major within span)."""
    xr = xT.reshape(HK, P, -1)
    cols = []
    for (c0, w), sps in zip(chunks, strips):
        for s0, sw in sps:
            a = c0 + s0
            cols.append(xr[:, :, a:a + sw].transpose(1, 0, 2)
                        .reshape(P, HK * sw))
    return np.ascontiguousarray(np.concatenate(cols, axis=1).astype(mdt))


def _unpack_y(img, chunks, C):
    """[P, HN*C] image -> [H, C]."""
    y = np.empty((H, C), np.float32)
    for c0, w in chunks:
        blk = np.asarray(img[:, HN * c0:HN * (c0 + w)], np.float32)
        y[:, c0:c0 + w] = blk.reshape(P, HN, w).transpose(1, 0, 2).reshape(H, w)
    return y


def _run(inputs, trace=False):
    global PRECISION
    from concourse.bass_utils import run_bass_kernel_spmd

    x, gate_w, W1, b1, W2, b2 = (inputs[k] for k in
                                 ("x", "gate_w", "W1", "b1", "W2", "b2"))
    x = np.asarray(x)
    Bb, S, Hd = x.shape
    assert Hd == H
    T = Bb * S
    xf = np.ascontiguousarray(x.reshape(T, Hd), dtype=np.float32)
    gate_w = np.asarray(gate_w, np.float32)

    # fp16 matmul operands need moderate dynamic range; fall back to
    # bf16 (full fp32 exponent range) if the data is far outside the
    # expected unit-scale regime.
    if PRECISION == "fp16":
        amax = max(float(np.abs(np.asarray(t)).max())
                   for t in (xf, W1, W2))
        if not np.isfinite(amax) or amax > 1e3:
            PRECISION = "bf16"
    mdt = _np_mdt()

    toks, ws = _route(xf, gate_w)
    nmax = max(len(t) for t in toks)
    # capacity = max expert load, no padding: with 2-byte elements every
    # row/offset is byte-even for any C, and each padded column would
    # cost 288 dead PE cycles.
    C = max(P, nmax)
    chunks = _chunks(C)
    strips = _strips(chunks)

    b1a = np.asarray(b1, np.float32)
    b2a = np.asarray(b2, np.float32)
    in_maps = []
    for e in range(E):
        n_e = len(toks[e])
        xT = np.zeros((H, C), np.float32)
        xT[:, :n_e] = xf[toks[e]].T
        cpk = np.concatenate([b1a[e].reshape(FM, P).T,
                              b2a[e].reshape(HN, P).T], axis=1)
        in_maps.append({
            "xgt": _pack_xg(xT, chunks, strips, mdt),
            "w1": _pack_w1(W1[e], mdt),
            "w2": _pack_w2(W2[e], mdt),
            "cpk": np.ascontiguousarray(cpk),
        })

    nc = _build_bass(C)

    kwargs = {}
    if trace:
        kwargs = dict(trace=True, trace_cores=list(range(N_CORES)))
    try:
        res = run_bass_kernel_spmd(nc, in_maps, core_ids=list(range(N_CORES)),
                                   **kwargs)
    except Exception:
        # One retry for transient device faults.
        time.sleep(5)
        res = run_bass_kernel_spmd(nc, in_maps, core_ids=list(range(N_CORES)),
                                   **kwargs)
    out = np.zeros((T, H), np.float32)
    for e in range(E):
        n_e = len(toks[e])
        ye = _unpack_y(res.results[e]["y"], chunks, C)  # [H, C] fp32
        out[toks[e]] += ws[e][:, None] * ye[:, :n_e].T
    return out.reshape(Bb, S, Hd), res


def kernel(x, gate_w, W1, b1, W2, b2):
    out, _ = _run({"x": x, "gate_w": gate_w, "W1": W1, "b1": b1,
                   "W2": W2, "b2": b2})
    return out.astype(np.asarray(x).dtype, copy=False)


# Exposed for test.py: run with profiling, return (output, BassKernelResults)
def kernel_profiled(x, gate_w, W1, b1, W2, b2):
    return _run({"x": x, "gate_w": gate_w, "W1": W1, "b1": b1,
                 "W2": W2, "b2": b2}, trace=True)
